# revision 27
# baseline (speedup 1.0000x reference)
"""MultiHeadAttention (qk-LayerNorm + RoPE) Trainium2 kernel, 8 NeuronCores.

Sharding: batch (4) x query-half (2x512 tokens), collective-free. Core c
handles batch c//2, query rows (c%2)*512 .. +512, ALL 16 heads. K/V
projections for the full 1024-token sequence are duplicated within each
batch pair; in exchange there are no collectives at all (the multi-device
rendezvous barrier plus ReduceScatter cost ~120us in the pair-parallel
variant). Each core writes its 512 output rows directly; the host
concatenates.

All matmuls run in bf16 (1 col/cycle on the PE, half the DMA/SBUF of
f32r). LayerNorm means are folded into Wq/Wk on the host (per-head
row-block mean subtraction makes projections zero-mean), so only the
variance is computed on device. D^-0.5 is folded into the q rope tables.
Scores are computed transposed ([s, t]) so the softmax denominator comes
free from a ones column appended to V; normalization uses one
reciprocal_approx_fast over head-stacked sums and a DMA partition
broadcast. o_proj packs head pairs (2x64 rows) for full 128-deep
contraction.
"""
import sys

for _p in ("/opt/trn_rl_repo", "/root/.axon_site", "/root/.axon_site/_ro/trn_rl_repo",
           "/root/.axon_site/_ro/pypackages"):
    if _p not in sys.path:
        sys.path.append(_p)

import numpy as np
import ml_dtypes

import concourse.bass as bass
import concourse.tile as tile
from concourse import bacc, mybir
from concourse.bass_utils import run_bass_kernel_spmd
from concourse.masks import make_identity

BF16_NP = ml_dtypes.bfloat16
F32 = mybir.dt.float32
BF16 = mybir.dt.bfloat16
P = 128
B, L, C, H, D = 4, 1024, 1024, 16, 64
LQ = L // 2          # query rows per core
NTQ = LQ // P        # 4 query token tiles
NT = L // P          # 8 key token tiles
NCK = C // P         # 8 contraction tiles
NPR = H // 2         # 8 head pairs
THETA = 50000.0
EPS = 1e-5

_NC_CACHE = {}


def _build_nc():
    nc = bacc.Bacc("TRN2", target_bir_lowering=False, debug=False, num_devices=8)

    xqT_d = nc.dram_tensor("xqT", [C, LQ], BF16, kind="ExternalInput")
    xrT_d = nc.dram_tensor("xrT", [C, LQ], BF16, kind="ExternalInput")
    wqT_d = nc.dram_tensor("wqT", [C, C], BF16, kind="ExternalInput")
    wkT_d = nc.dram_tensor("wkT", [C, C], BF16, kind="ExternalInput")
    wvT_d = nc.dram_tensor("wvT", [C, C], BF16, kind="ExternalInput")
    woP_d = nc.dram_tensor("woP", [NPR, P, C], BF16, kind="ExternalInput")
    aq_d = nc.dram_tensor("aq", [LQ, D], BF16, kind="ExternalInput")
    bq_d = nc.dram_tensor("bq", [LQ, D], BF16, kind="ExternalInput")
    ak_d = nc.dram_tensor("ak", [L, D], BF16, kind="ExternalInput")
    bk_d = nc.dram_tensor("bk", [L, D], BF16, kind="ExternalInput")
    out_d = nc.dram_tensor("out", [LQ, C], F32, kind="ExternalOutput")

    with tile.TileContext(nc) as tc:
        with (
            tc.tile_pool(name="const", bufs=1) as constp,
            tc.tile_pool(name="w", bufs=1) as wpool,
            tc.tile_pool(name="big", bufs=1) as bigp,
            tc.tile_pool(name="xt", bufs=2) as xtp,
            tc.tile_pool(name="stg", bufs=2) as stgp,
            tc.tile_pool(name="stat", bufs=2) as statp,
            tc.tile_pool(name="exp", bufs=2) as expp,
            tc.tile_pool(name="fin", bufs=2) as finp,
            tc.tile_pool(name="dram", bufs=1, space="DRAM") as dramp,
        ):
            ident = constp.tile([P, P], BF16)
            make_identity(nc, ident)
            eps_t = constp.tile([P, 1], F32)
            nc.vector.memset(eps_t[:], EPS)
            ones_row = constp.tile([1, D], BF16)
            nc.vector.memset(ones_row[:], 1.0)
            one_f32 = constp.tile([1, 1], F32)
            nc.vector.memset(one_f32[:], 1.0)

            aq_t = constp.tile([P, NTQ, D], BF16)
            nc.sync.dma_start(aq_t[:], aq_d.ap().rearrange("(t p) d -> p t d", p=P))
            bq_t = constp.tile([P, NTQ, D], BF16)
            nc.sync.dma_start(bq_t[:], bq_d.ap().rearrange("(t p) d -> p t d", p=P))
            ak_t = constp.tile([P, NT, D], BF16)
            nc.sync.dma_start(ak_t[:], ak_d.ap().rearrange("(t p) d -> p t d", p=P))
            bk_t = constp.tile([P, NT, D], BF16)
            nc.sync.dma_start(bk_t[:], bk_d.ap().rearrange("(t p) d -> p t d", p=P))

            # weight tiles; q weights first (halves) so phase Q starts early,
            # consolidated DMAs for bandwidth
            wq_t = wpool.tile([P, NCK, C], BF16, tag="wq", name="wq")
            wk_t = wpool.tile([P, NCK, C], BF16, tag="wk", name="wk")
            wv_t = wpool.tile([P, NCK, C], BF16, tag="wv", name="wv")
            for t_, d_ in ((wq_t, wqT_d), (wk_t, wkT_d), (wv_t, wvT_d)):
                for hf in range(2):
                    nc.sync.dma_start(
                        t_[:, bass.ts(hf, NCK // 2), :],
                        d_.ap().rearrange("(k p) o -> p k o", p=P)[:, bass.ts(hf, NCK // 2), :])
            wo_t = wpool.tile([P, NPR, C], BF16, tag="wo", name="wo")
            nc.sync.dma_start(wo_t[:], woP_d.ap().rearrange("r p o -> p r o"))

            # v with a ones column appended per head: [s, j, h, 65]
            v_sb = bigp.tile([P, NT, H, D + 1], BF16)
            nc.gpsimd.memset(v_sb[:, :, :, D:D + 1], 1.0)

            qT_pack = bigp.tile([P, NPR, LQ], BF16)
            kT_pack = bigp.tile([P, NPR, L], BF16)
            ctxT = bigp.tile([P, NPR, LQ], BF16)

            def ln_rope(ps, a_t, b_t, ti, dst_pack, pst_pool):
                """psum [t,1024] f32 -> LN(var-only)+rope -> transpose into
                dst_pack[:, pr, ti*128:...]. Returns nothing."""
                qs = stgp.tile([P, H, D], BF16, tag="stg")
                nc.scalar.copy(qs[:], ps[:].rearrange("p (h d) -> p h d", d=D))
                sq = stgp.tile([P, H, D], BF16, tag="sq")
                nc.scalar.square(sq[:], qs[:])
                ss = statp.tile([P, H], F32, tag="ss")
                nc.vector.reduce_sum(ss[:], sq[:], axis=mybir.AxisListType.X)
                std = statp.tile([P, H], F32, tag="std")
                nc.scalar.activation(std[:], ss[:], mybir.ActivationFunctionType.Sqrt,
                                     bias=eps_t[:], scale=1.0 / D)
                inv = statp.tile([P, H], F32, tag="inv")
                nc.vector.reciprocal(inv[:], std[:])
                invb = statp.tile([P, H], BF16, tag="invb")
                nc.vector.tensor_copy(invb[:], inv[:])

                a_b = a_t[:, ti, :].rearrange("p d -> p () d").to_broadcast((P, H, D))
                r = stgp.tile([P, H, D], BF16, tag="r")
                nc.vector.tensor_mul(r[:], qs[:], a_b)
                r2 = stgp.tile([P, H, D], BF16, tag="r2")
                h_ = D // 2
                nc.vector.tensor_mul(
                    r2[:, :, 0:h_], qs[:, :, h_:D],
                    b_t[:, ti, 0:h_].rearrange("p d -> p () d").to_broadcast((P, H, h_)))
                nc.vector.tensor_mul(
                    r2[:, :, h_:D], qs[:, :, 0:h_],
                    b_t[:, ti, h_:D].rearrange("p d -> p () d").to_broadcast((P, H, h_)))
                nc.vector.tensor_add(r[:], r[:], r2[:])
                nc.vector.tensor_mul(
                    r[:], r[:],
                    invb[:].rearrange("p h -> p h ()").to_broadcast((P, H, D)))

                for grp in range(2):
                    pst = pst_pool.tile([P, 4, P], BF16, tag="pst")
                    for q4 in range(4):
                        pr = grp * 4 + q4
                        nc.tensor.transpose(
                            pst[:, q4, :],
                            r[:, 2 * pr:2 * pr + 2, :].rearrange("p h d -> p (h d)"),
                            ident[:])
                    nc.vector.tensor_copy(
                        dst_pack[:, 4 * grp:4 * grp + 4, bass.ts(ti, P)], pst[:])

            # ---------------- Phase Q: q projection + LN + RoPE -------------
            with tc.tile_pool(name="psq", bufs=2, space="PSUM") as psqp, \
                 tc.tile_pool(name="pstq", bufs=2, space="PSUM") as pstqp:
                xq_tiles = []
                for ti in range(NTQ):
                    xt = xtp.tile([P, NCK, P], BF16, tag=f"xq{ti}")
                    nc.scalar.dma_start(
                        xt[:],
                        xqT_d.ap().rearrange("(k p) t -> p k t", p=P)[:, :, bass.ts(ti, P)])
                    xq_tiles.append(xt)
                    psq = psqp.tile([P, C], F32)
                    for ch in range(2):
                        for ck in range(NCK):
                            nc.tensor.matmul(psq[:, bass.ts(ch, 512)],
                                             xt[:, ck, :],
                                             wq_t[:, ck, bass.ts(ch, 512)],
                                             start=(ck == 0), stop=(ck == NCK - 1))
                    ln_rope(psq, aq_t, bq_t, ti, qT_pack, pstqp)

            # ---------------- Phase KV: k/v projection + LN + RoPE ----------
            with tc.tile_pool(name="psk", bufs=2, space="PSUM") as pskp, \
                 tc.tile_pool(name="psv", bufs=1, space="PSUM") as psvp, \
                 tc.tile_pool(name="pstk", bufs=2, space="PSUM") as pstkp:
                for ti in range(NT):
                    # k/v tiles 0-3 are this core's query half: reuse the
                    # x tiles already in SBUF; 4-7 come from the other half
                    if ti < NTQ:
                        xt = xq_tiles[ti]
                    else:
                        xt = xtp.tile([P, NCK, P], BF16, tag="xk")
                        nc.scalar.dma_start(
                            xt[:],
                            xrT_d.ap().rearrange("(k p) t -> p k t", p=P)[:, :, bass.ts(ti - NTQ, P)])
                    psk = pskp.tile([P, C], F32)
                    psv = psvp.tile([P, C], F32)
                    for ps_, w_ in ((psk, wk_t), (psv, wv_t)):
                        for ch in range(2):
                            for ck in range(NCK):
                                nc.tensor.matmul(ps_[:, bass.ts(ch, 512)],
                                                 xt[:, ck, :],
                                                 w_[:, ck, bass.ts(ch, 512)],
                                                 start=(ck == 0), stop=(ck == NCK - 1))
                    nc.scalar.copy(
                        v_sb[:, ti, :, 0:D],
                        psv[:].rearrange("p (h d) -> p h d", d=D))
                    ln_rope(psk, ak_t, bk_t, ti, kT_pack, pstkp)

            # ---------------- Phase ATT: attention, head pair per round -----
            with tc.tile_pool(name="pss", bufs=2, space="PSUM") as pssp, \
                 tc.tile_pool(name="psc", bufs=1, space="PSUM") as pscp, \
                 tc.tile_pool(name="psst", bufs=1, space="PSUM") as psstp, \
                 tc.tile_pool(name="psrb", bufs=1, space="PSUM") as psrbp:
                for pr in range(NPR):
                    psc = [pscp.tile([D + 1, LQ], F32, tag=f"c{i}",
                                     name=f"psc{pr}_{i}") for i in range(2)]
                    for j in range(NT):
                        pss = pssp.tile([P, 2, LQ], F32, tag="pss")
                        for i in range(2):
                            lo = i * D
                            nc.tensor.matmul(
                                pss[:, i, :],
                                kT_pack[lo:lo + D, pr, bass.ts(j, P)],
                                qT_pack[lo:lo + D, pr, :],
                                start=True, stop=True)
                        expT = expp.tile([P, 2, LQ], BF16, tag="expT")
                        nc.scalar.activation(expT[:], pss[:],
                                             mybir.ActivationFunctionType.Exp)
                        for i in range(2):
                            nc.tensor.matmul(
                                psc[i][:],
                                v_sb[:, j, 2 * pr + i, :],
                                expT[:, i, :],
                                start=(j == 0), stop=(j == NT - 1))
                    # drain pair: unnormalized ctx (2 heads packed on 128p)
                    # and sums; reciprocal runs 128-way via transposed layout
                    smq = finp.tile([1, 2, LQ], F32, tag="smq")
                    for i in range(2):
                        lo = i * D
                        nc.vector.tensor_copy(
                            ctxT[lo:lo + D, pr, :], psc[i][0:D, :])
                        nc.vector.tensor_copy(smq[0:1, i, :], psc[i][D:D + 1, :])
                    sumsT = psstp.tile([P, 2, 4], F32, tag="sumsT")
                    for i in range(2):
                        for c in range(4):
                            nc.tensor.transpose(
                                sumsT[:, i, c:c + 1],
                                smq[0:1, i, bass.ts(c, P)], one_f32[:])
                    rbT = finp.tile([P, 2, 4], BF16, tag="rbT")
                    with nc.allow_low_precision(reason="softmax recip bf16"):
                        nc.vector.reciprocal(rbT[:], sumsT[:])
                    rbrow_ps = psrbp.tile([1, 2, LQ], BF16, tag="rbrow")
                    for i in range(2):
                        for c in range(4):
                            nc.tensor.transpose(
                                rbrow_ps[0:1, i, bass.ts(c, P)],
                                rbT[:, i, c:c + 1], ident[:])
                    rbrow = finp.tile([1, 2, LQ], BF16, tag="rbrow_sb")
                    nc.vector.tensor_copy(rbrow[:], rbrow_ps[:])
                    # broadcast 1/sums across 64 partitions via K=1 matmul.
                    # rb_ps shares the rbrow bank (sequential use) so the psc
                    # banks free right after the drain copies — the next
                    # pair's ctx accumulation starts without waiting on the
                    # normalize chain.
                    for i in range(2):
                        lo = i * D
                        rb_ps = psrbp.tile([D, LQ], F32, tag="rbrow",
                                           name=f"rbps{pr}_{i}")
                        nc.tensor.matmul(rb_ps[:], ones_row[:],
                                         rbrow[0:1, i, :], start=True, stop=True)
                        nc.vector.tensor_mul(ctxT[lo:lo + D, pr, :],
                                             ctxT[lo:lo + D, pr, :],
                                             rb_ps[:])

            # ---------------- Phase O: output projection --------------------
            with tc.tile_pool(name="pso", bufs=2, space="PSUM") as psop:
                for ti in range(NTQ):
                    pso = psop.tile([P, C], F32)
                    for pr in range(NPR):
                        for ch in range(2):
                            nc.tensor.matmul(
                                pso[:, bass.ts(ch, 512)],
                                ctxT[:, pr, bass.ts(ti, P)],
                                wo_t[:, pr, bass.ts(ch, 512)],
                                start=(pr == 0), stop=(pr == NPR - 1))
                    out_sb = finp.tile([P, C], F32, tag="out")
                    nc.vector.tensor_copy(out_sb[:], pso[:])
                    nc.sync.dma_start(out_d.ap()[bass.ts(ti, P), :], out_sb[:])

    nc.compile()
    return nc


def _rope_tables(w, b, length, scale):
    """A[t,d], B[t,d] with rotate-half sign and LN weight folded in."""
    inv_freq = 1.0 / THETA ** (np.arange(0, D, 2, dtype=np.float64) / D)
    freqs = np.arange(length, dtype=np.float64)[:, None] * inv_freq[None, :]
    freqs = np.concatenate([freqs, freqs], axis=1)
    cos, sin = np.cos(freqs), np.sin(freqs)
    w = w.astype(np.float64)
    w_rot = np.concatenate([w[D // 2:], w[:D // 2]])
    sgn = np.concatenate([-np.ones(D // 2), np.ones(D // 2)])
    A = (cos * w[None, :] * scale).astype(BF16_NP)
    Bt = (sin * w_rot[None, :] * sgn[None, :] * scale).astype(BF16_NP)
    if np.any(b != 0):
        raise NotImplementedError("nonzero qk-norm bias not supported")
    return A, Bt


def _fold_mean(W):
    """Remove per-head row-block mean: projections become zero-mean."""
    W = W.astype(np.float64).copy()
    for h in range(H):
        W[h * D:(h + 1) * D, :] -= W[h * D:(h + 1) * D, :].mean(0, keepdims=True)
    return W


def kernel(**inputs):
    x = np.asarray(inputs["q"], dtype=np.float32)
    Wq = np.asarray(inputs["Wq"], dtype=np.float32)
    Wk = np.asarray(inputs["Wk"], dtype=np.float32)
    Wv = np.asarray(inputs["Wv"], dtype=np.float32)
    Wo = np.asarray(inputs["Wo"], dtype=np.float32)
    bo = np.asarray(inputs["bo"], dtype=np.float32)
    assert not np.any(bo != 0), "nonzero output bias not supported"

    Aq, Bq = _rope_tables(np.asarray(inputs["qn_w"], np.float32),
                          np.asarray(inputs["qn_b"], np.float32), L, D ** -0.5)
    Ak, Bk = _rope_tables(np.asarray(inputs["kn_w"], np.float32),
                          np.asarray(inputs["kn_b"], np.float32), L, 1.0)

    wqT = np.ascontiguousarray(_fold_mean(Wq).T).astype(BF16_NP)
    wkT = np.ascontiguousarray(_fold_mean(Wk).T).astype(BF16_NP)
    wvT = np.ascontiguousarray(Wv.T.astype(np.float64)).astype(BF16_NP)
    # o_proj pair-packed: WoT rows grouped (pair, 2 heads x 64)
    woP = np.ascontiguousarray(
        Wo.T.astype(np.float64).reshape(NPR, P, C)).astype(BF16_NP)

    if "nc" not in _NC_CACHE:
        _NC_CACHE["nc"] = _build_nc()
    nc = _NC_CACHE["nc"]

    in_maps = []
    for c in range(8):
        b_, half = c // 2, c % 2
        xT = np.ascontiguousarray(x[b_].T.astype(np.float64)).astype(BF16_NP)
        own = slice(half * LQ, (half + 1) * LQ)
        oth = slice((1 - half) * LQ, (2 - half) * LQ)
        # k/v sequence order per core: [own half, other half] — attention
        # sums are order-independent as long as the k rope tables match
        in_maps.append({
            "xqT": np.ascontiguousarray(xT[:, own]),
            "xrT": np.ascontiguousarray(xT[:, oth]),
            "wqT": wqT, "wkT": wkT, "wvT": wvT, "woP": woP,
            "aq": np.ascontiguousarray(Aq[own]),
            "bq": np.ascontiguousarray(Bq[own]),
            "ak": np.ascontiguousarray(np.concatenate([Ak[own], Ak[oth]], 0)),
            "bk": np.ascontiguousarray(np.concatenate([Bk[own], Bk[oth]], 0)),
        })

    res = run_bass_kernel_spmd(nc, in_maps, core_ids=list(range(8)))
    out = np.empty((B, L, C), dtype=np.float32)
    for c in range(8):
        b_, half = c // 2, c % 2
        out[b_, half * LQ:(half + 1) * LQ] = res.results[c]["out"]
    return out


# revision 28
# speedup vs baseline: 1.1645x; 1.1645x over previous
"""MultiHeadAttention (qk-LayerNorm + RoPE) Trainium2 kernel, 8 NeuronCores.

Sharding: batch (4) x query-half (2x512 tokens), collective-free. Core c
handles batch c//2, query rows (c%2)*512 .. +512, ALL 16 heads. K/V
projections for the full 1024-token sequence are duplicated within each
batch pair; in exchange there are no collectives at all (the multi-device
rendezvous barrier plus ReduceScatter cost ~120us in the pair-parallel
variant). Each core writes its 512 output rows directly; the host
concatenates.

All matmuls run in bf16 (1 col/cycle on the PE, half the DMA/SBUF of
f32r). LayerNorm means are folded into Wq/Wk on the host (per-head
row-block mean subtraction makes projections zero-mean), so only the
variance is computed on device. D^-0.5 is folded into the q rope tables.
Scores are computed transposed ([s, t]) so the softmax denominator comes
free from a ones column appended to V; normalization uses one
reciprocal_approx_fast over head-stacked sums and a DMA partition
broadcast. o_proj packs head pairs (2x64 rows) for full 128-deep
contraction.
"""
import sys

for _p in ("/opt/trn_rl_repo", "/root/.axon_site", "/root/.axon_site/_ro/trn_rl_repo",
           "/root/.axon_site/_ro/pypackages"):
    if _p not in sys.path:
        sys.path.append(_p)

import numpy as np
import ml_dtypes

import concourse.bass as bass
import concourse.tile as tile
from concourse import bacc, mybir
from concourse.bass_utils import run_bass_kernel_spmd
from concourse.masks import make_identity

BF16_NP = ml_dtypes.bfloat16
F32 = mybir.dt.float32
BF16 = mybir.dt.bfloat16
P = 128
B, L, C, H, D = 4, 1024, 1024, 16, 64
LQ = L // 2          # query rows per core
NTQ = LQ // P        # 4 query token tiles
NT = L // P          # 8 key token tiles
NCK = C // P         # 8 contraction tiles
NPR = H // 2         # 8 head pairs
THETA = 50000.0
EPS = 1e-5

_NC_CACHE = {}


def _build_nc():
    nc = bacc.Bacc("TRN2", target_bir_lowering=False, debug=False, num_devices=8)

    xqT_d = nc.dram_tensor("xqT", [C, LQ], BF16, kind="ExternalInput")
    xrT_d = nc.dram_tensor("xrT", [C, LQ], BF16, kind="ExternalInput")
    wqT_d = nc.dram_tensor("wqT", [C, C], BF16, kind="ExternalInput")
    wkT_d = nc.dram_tensor("wkT", [C, C], BF16, kind="ExternalInput")
    wvT_d = nc.dram_tensor("wvT", [C, C], BF16, kind="ExternalInput")
    woP_d = nc.dram_tensor("woP", [NPR, P, C], BF16, kind="ExternalInput")
    aq_d = nc.dram_tensor("aq", [LQ, D], BF16, kind="ExternalInput")
    bq_d = nc.dram_tensor("bq", [LQ, D], BF16, kind="ExternalInput")
    ak_d = nc.dram_tensor("ak", [L, D], BF16, kind="ExternalInput")
    bk_d = nc.dram_tensor("bk", [L, D], BF16, kind="ExternalInput")
    out_d = nc.dram_tensor("out", [LQ, C], F32, kind="ExternalOutput")

    with tile.TileContext(nc) as tc:
        with (
            tc.tile_pool(name="const", bufs=1) as constp,
            tc.tile_pool(name="w", bufs=1) as wpool,
            tc.tile_pool(name="big", bufs=1) as bigp,
            tc.tile_pool(name="xt", bufs=2) as xtp,
            tc.tile_pool(name="stg", bufs=2) as stgp,
            tc.tile_pool(name="stat", bufs=2) as statp,
            tc.tile_pool(name="exp", bufs=2) as expp,
            tc.tile_pool(name="fin", bufs=2) as finp,
            tc.tile_pool(name="dram", bufs=1, space="DRAM") as dramp,
        ):
            ident = constp.tile([P, P], BF16)
            make_identity(nc, ident)
            eps_t = constp.tile([P, 1], F32)
            nc.vector.memset(eps_t[:], EPS)
            ones_row = constp.tile([1, D], BF16)
            nc.vector.memset(ones_row[:], 1.0)
            one_f32 = constp.tile([1, 1], F32)
            nc.vector.memset(one_f32[:], 1.0)

            aq_t = constp.tile([P, NTQ, D], BF16)
            nc.sync.dma_start(aq_t[:], aq_d.ap().rearrange("(t p) d -> p t d", p=P))
            bq_t = constp.tile([P, NTQ, D], BF16)
            nc.sync.dma_start(bq_t[:], bq_d.ap().rearrange("(t p) d -> p t d", p=P))
            ak_t = constp.tile([P, NT, D], BF16)
            nc.sync.dma_start(ak_t[:], ak_d.ap().rearrange("(t p) d -> p t d", p=P))
            bk_t = constp.tile([P, NT, D], BF16)
            nc.sync.dma_start(bk_t[:], bk_d.ap().rearrange("(t p) d -> p t d", p=P))

            # weight tiles; q weights first so phase Q starts early.
            # per-ck DMAs: big consolidated transfers measured SLOWER
            # (tensor busy +32us — SBUF write bursts stall xbus streaming)
            wq_t = wpool.tile([P, NCK, C], BF16, tag="wq", name="wq")
            wk_t = wpool.tile([P, NCK, C], BF16, tag="wk", name="wk")
            wv_t = wpool.tile([P, NCK, C], BF16, tag="wv", name="wv")
            for t_, d_ in ((wq_t, wqT_d), (wk_t, wkT_d), (wv_t, wvT_d)):
                for ck in range(NCK):
                    nc.sync.dma_start(
                        t_[:, ck, :],
                        d_.ap().rearrange("(k p) o -> p k o", p=P)[:, ck, :])
            wo_t = wpool.tile([P, NPR, C], BF16, tag="wo", name="wo")
            for pr in range(NPR):
                nc.sync.dma_start(wo_t[:, pr, :], woP_d.ap()[pr])

            # v with a ones column appended per head: [s, j, h, 65]
            v_sb = bigp.tile([P, NT, H, D + 1], BF16)
            nc.gpsimd.memset(v_sb[:, :, :, D:D + 1], 1.0)

            qT_pack = bigp.tile([P, NPR, LQ], BF16)
            kT_pack = bigp.tile([P, NPR, L], BF16)
            ctxT = bigp.tile([P, NPR, LQ], BF16)

            def ln_rope(ps, a_t, b_t, ti, dst_pack, pst_pool):
                """psum [t,1024] f32 -> LN(var-only)+rope -> transpose into
                dst_pack[:, pr, ti*128:...]. Returns nothing."""
                qs = stgp.tile([P, H, D], BF16, tag="stg")
                nc.scalar.copy(qs[:], ps[:].rearrange("p (h d) -> p h d", d=D))
                sq = stgp.tile([P, H, D], BF16, tag="sq")
                nc.scalar.square(sq[:], qs[:])
                ss = statp.tile([P, H], F32, tag="ss")
                nc.vector.reduce_sum(ss[:], sq[:], axis=mybir.AxisListType.X)
                std = statp.tile([P, H], F32, tag="std")
                nc.scalar.activation(std[:], ss[:], mybir.ActivationFunctionType.Sqrt,
                                     bias=eps_t[:], scale=1.0 / D)
                inv = statp.tile([P, H], F32, tag="inv")
                nc.vector.reciprocal(inv[:], std[:])
                invb = statp.tile([P, H], BF16, tag="invb")
                nc.vector.tensor_copy(invb[:], inv[:])

                a_b = a_t[:, ti, :].rearrange("p d -> p () d").to_broadcast((P, H, D))
                r = stgp.tile([P, H, D], BF16, tag="r")
                nc.vector.tensor_mul(r[:], qs[:], a_b)
                r2 = stgp.tile([P, H, D], BF16, tag="r2")
                h_ = D // 2
                nc.vector.tensor_mul(
                    r2[:, :, 0:h_], qs[:, :, h_:D],
                    b_t[:, ti, 0:h_].rearrange("p d -> p () d").to_broadcast((P, H, h_)))
                nc.vector.tensor_mul(
                    r2[:, :, h_:D], qs[:, :, 0:h_],
                    b_t[:, ti, h_:D].rearrange("p d -> p () d").to_broadcast((P, H, h_)))
                nc.vector.tensor_add(r[:], r[:], r2[:])
                nc.vector.tensor_mul(
                    r[:], r[:],
                    invb[:].rearrange("p h -> p h ()").to_broadcast((P, H, D)))

                for grp in range(2):
                    pst = pst_pool.tile([P, 4, P], BF16, tag="pst")
                    for q4 in range(4):
                        pr = grp * 4 + q4
                        nc.tensor.transpose(
                            pst[:, q4, :],
                            r[:, 2 * pr:2 * pr + 2, :].rearrange("p h d -> p (h d)"),
                            ident[:])
                    nc.vector.tensor_copy(
                        dst_pack[:, 4 * grp:4 * grp + 4, bass.ts(ti, P)], pst[:])

            # ---------------- Phase Q: q projection + LN + RoPE -------------
            with tc.tile_pool(name="psq", bufs=2, space="PSUM") as psqp, \
                 tc.tile_pool(name="pstq", bufs=2, space="PSUM") as pstqp:
                xq_tiles = []
                for ti in range(NTQ):
                    xt = xtp.tile([P, NCK, P], BF16, tag=f"xq{ti}")
                    nc.scalar.dma_start(
                        xt[:],
                        xqT_d.ap().rearrange("(k p) t -> p k t", p=P)[:, :, bass.ts(ti, P)])
                    xq_tiles.append(xt)
                    psq = psqp.tile([P, C], F32)
                    for ch in range(2):
                        for ck in range(NCK):
                            nc.tensor.matmul(psq[:, bass.ts(ch, 512)],
                                             xt[:, ck, :],
                                             wq_t[:, ck, bass.ts(ch, 512)],
                                             start=(ck == 0), stop=(ck == NCK - 1))
                    ln_rope(psq, aq_t, bq_t, ti, qT_pack, pstqp)

            # ---------------- Phase KV: k/v projection + LN + RoPE ----------
            with tc.tile_pool(name="psk", bufs=2, space="PSUM") as pskp, \
                 tc.tile_pool(name="psv", bufs=1, space="PSUM") as psvp, \
                 tc.tile_pool(name="pstk", bufs=2, space="PSUM") as pstkp:
                for ti in range(NT):
                    # k/v tiles 0-3 are this core's query half: reuse the
                    # x tiles already in SBUF; 4-7 come from the other half
                    if ti < NTQ:
                        xt = xq_tiles[ti]
                    else:
                        xt = xtp.tile([P, NCK, P], BF16, tag="xk")
                        nc.scalar.dma_start(
                            xt[:],
                            xrT_d.ap().rearrange("(k p) t -> p k t", p=P)[:, :, bass.ts(ti - NTQ, P)])
                    psk = pskp.tile([P, C], F32)
                    psv = psvp.tile([P, C], F32)
                    for ps_, w_ in ((psk, wk_t), (psv, wv_t)):
                        for ch in range(2):
                            for ck in range(NCK):
                                nc.tensor.matmul(ps_[:, bass.ts(ch, 512)],
                                                 xt[:, ck, :],
                                                 w_[:, ck, bass.ts(ch, 512)],
                                                 start=(ck == 0), stop=(ck == NCK - 1))
                    nc.scalar.copy(
                        v_sb[:, ti, :, 0:D],
                        psv[:].rearrange("p (h d) -> p h d", d=D))
                    ln_rope(psk, ak_t, bk_t, ti, kT_pack, pstkp)

            # ---------------- Phase ATT: attention, head pair per round -----
            with tc.tile_pool(name="pss", bufs=2, space="PSUM") as pssp, \
                 tc.tile_pool(name="psc", bufs=1, space="PSUM") as pscp, \
                 tc.tile_pool(name="psst", bufs=1, space="PSUM") as psstp, \
                 tc.tile_pool(name="psrb", bufs=1, space="PSUM") as psrbp:
                for pr in range(NPR):
                    psc = [pscp.tile([D + 1, LQ], F32, tag=f"c{i}",
                                     name=f"psc{pr}_{i}") for i in range(2)]
                    for j in range(NT):
                        pss = pssp.tile([P, 2, LQ], F32, tag="pss")
                        for i in range(2):
                            lo = i * D
                            nc.tensor.matmul(
                                pss[:, i, :],
                                kT_pack[lo:lo + D, pr, bass.ts(j, P)],
                                qT_pack[lo:lo + D, pr, :],
                                start=True, stop=True)
                        expT = expp.tile([P, 2, LQ], BF16, tag="expT")
                        nc.scalar.activation(expT[:], pss[:],
                                             mybir.ActivationFunctionType.Exp)
                        for i in range(2):
                            nc.tensor.matmul(
                                psc[i][:],
                                v_sb[:, j, 2 * pr + i, :],
                                expT[:, i, :],
                                start=(j == 0), stop=(j == NT - 1))
                    # drain pair: unnormalized ctx (2 heads packed on 128p)
                    # and sums; reciprocal runs 128-way via transposed layout
                    smq = finp.tile([1, 2, LQ], F32, tag="smq")
                    for i in range(2):
                        lo = i * D
                        nc.vector.tensor_copy(
                            ctxT[lo:lo + D, pr, :], psc[i][0:D, :])
                        nc.vector.tensor_copy(smq[0:1, i, :], psc[i][D:D + 1, :])
                    sumsT = psstp.tile([P, 2, 4], F32, tag="sumsT")
                    for i in range(2):
                        for c in range(4):
                            nc.tensor.transpose(
                                sumsT[:, i, c:c + 1],
                                smq[0:1, i, bass.ts(c, P)], one_f32[:])
                    rbT = finp.tile([P, 2, 4], BF16, tag="rbT")
                    with nc.allow_low_precision(reason="softmax recip bf16"):
                        nc.vector.reciprocal(rbT[:], sumsT[:])
                    rbrow_ps = psrbp.tile([1, 2, LQ], BF16, tag="rbrow")
                    for i in range(2):
                        for c in range(4):
                            nc.tensor.transpose(
                                rbrow_ps[0:1, i, bass.ts(c, P)],
                                rbT[:, i, c:c + 1], ident[:])
                    rbrow = finp.tile([1, 2, LQ], BF16, tag="rbrow_sb")
                    nc.vector.tensor_copy(rbrow[:], rbrow_ps[:])
                    # broadcast 1/sums across 64 partitions via K=1 matmul.
                    # rb_ps shares the rbrow bank (sequential use) so the psc
                    # banks free right after the drain copies — the next
                    # pair's ctx accumulation starts without waiting on the
                    # normalize chain.
                    for i in range(2):
                        lo = i * D
                        rb_ps = psrbp.tile([D, LQ], F32, tag="rbrow",
                                           name=f"rbps{pr}_{i}")
                        nc.tensor.matmul(rb_ps[:], ones_row[:],
                                         rbrow[0:1, i, :], start=True, stop=True)
                        nc.vector.tensor_mul(ctxT[lo:lo + D, pr, :],
                                             ctxT[lo:lo + D, pr, :],
                                             rb_ps[:])

            # ---------------- Phase O: output projection --------------------
            with tc.tile_pool(name="pso", bufs=2, space="PSUM") as psop:
                for ti in range(NTQ):
                    pso = psop.tile([P, C], F32)
                    for pr in range(NPR):
                        for ch in range(2):
                            nc.tensor.matmul(
                                pso[:, bass.ts(ch, 512)],
                                ctxT[:, pr, bass.ts(ti, P)],
                                wo_t[:, pr, bass.ts(ch, 512)],
                                start=(pr == 0), stop=(pr == NPR - 1))
                    out_sb = finp.tile([P, C], F32, tag="out")
                    nc.vector.tensor_copy(out_sb[:], pso[:])
                    nc.sync.dma_start(out_d.ap()[bass.ts(ti, P), :], out_sb[:])

    nc.compile()
    return nc


def _rope_tables(w, b, length, scale):
    """A[t,d], B[t,d] with rotate-half sign and LN weight folded in."""
    inv_freq = 1.0 / THETA ** (np.arange(0, D, 2, dtype=np.float64) / D)
    freqs = np.arange(length, dtype=np.float64)[:, None] * inv_freq[None, :]
    freqs = np.concatenate([freqs, freqs], axis=1)
    cos, sin = np.cos(freqs), np.sin(freqs)
    w = w.astype(np.float64)
    w_rot = np.concatenate([w[D // 2:], w[:D // 2]])
    sgn = np.concatenate([-np.ones(D // 2), np.ones(D // 2)])
    A = (cos * w[None, :] * scale).astype(BF16_NP)
    Bt = (sin * w_rot[None, :] * sgn[None, :] * scale).astype(BF16_NP)
    if np.any(b != 0):
        raise NotImplementedError("nonzero qk-norm bias not supported")
    return A, Bt


def _fold_mean(W):
    """Remove per-head row-block mean: projections become zero-mean."""
    W = W.astype(np.float64).copy()
    for h in range(H):
        W[h * D:(h + 1) * D, :] -= W[h * D:(h + 1) * D, :].mean(0, keepdims=True)
    return W


def kernel(**inputs):
    x = np.asarray(inputs["q"], dtype=np.float32)
    Wq = np.asarray(inputs["Wq"], dtype=np.float32)
    Wk = np.asarray(inputs["Wk"], dtype=np.float32)
    Wv = np.asarray(inputs["Wv"], dtype=np.float32)
    Wo = np.asarray(inputs["Wo"], dtype=np.float32)
    bo = np.asarray(inputs["bo"], dtype=np.float32)
    assert not np.any(bo != 0), "nonzero output bias not supported"

    Aq, Bq = _rope_tables(np.asarray(inputs["qn_w"], np.float32),
                          np.asarray(inputs["qn_b"], np.float32), L, D ** -0.5)
    Ak, Bk = _rope_tables(np.asarray(inputs["kn_w"], np.float32),
                          np.asarray(inputs["kn_b"], np.float32), L, 1.0)

    wqT = np.ascontiguousarray(_fold_mean(Wq).T).astype(BF16_NP)
    wkT = np.ascontiguousarray(_fold_mean(Wk).T).astype(BF16_NP)
    wvT = np.ascontiguousarray(Wv.T.astype(np.float64)).astype(BF16_NP)
    # o_proj pair-packed: WoT rows grouped (pair, 2 heads x 64)
    woP = np.ascontiguousarray(
        Wo.T.astype(np.float64).reshape(NPR, P, C)).astype(BF16_NP)

    if "nc" not in _NC_CACHE:
        _NC_CACHE["nc"] = _build_nc()
    nc = _NC_CACHE["nc"]

    in_maps = []
    for c in range(8):
        b_, half = c // 2, c % 2
        xT = np.ascontiguousarray(x[b_].T.astype(np.float64)).astype(BF16_NP)
        own = slice(half * LQ, (half + 1) * LQ)
        oth = slice((1 - half) * LQ, (2 - half) * LQ)
        # k/v sequence order per core: [own half, other half] — attention
        # sums are order-independent as long as the k rope tables match
        in_maps.append({
            "xqT": np.ascontiguousarray(xT[:, own]),
            "xrT": np.ascontiguousarray(xT[:, oth]),
            "wqT": wqT, "wkT": wkT, "wvT": wvT, "woP": woP,
            "aq": np.ascontiguousarray(Aq[own]),
            "bq": np.ascontiguousarray(Bq[own]),
            "ak": np.ascontiguousarray(np.concatenate([Ak[own], Ak[oth]], 0)),
            "bk": np.ascontiguousarray(np.concatenate([Bk[own], Bk[oth]], 0)),
        })

    res = run_bass_kernel_spmd(nc, in_maps, core_ids=list(range(8)))
    out = np.empty((B, L, C), dtype=np.float32)
    for c in range(8):
        b_, half = c // 2, c % 2
        out[b_, half * LQ:(half + 1) * LQ] = res.results[c]["out"]
    return out
